# revision 1
# baseline (speedup 1.0000x reference)
"""Trainium2 Bass kernel for nn_AttnMatching.

Reference computes:
    emb = emb_table[1:L+1]                      # [L, D]
    attn = einsum('ld,ntd->nlt', emb, self_attn)
    out  = einsum('nlt,t->nl', attn, value_w[0])

Reassociated (identical math, fp32):
    ctx[n, d] = sum_t value_w[t] * self_attn[n, t, d]    # [N, D]  (tiny)
    out[n, l] = sum_d ctx[n, d] * emb[l, d]              # [N, L]

This is memory-bound: the dominant traffic is streaming the 25.6 MB
embedding table. Sharding: vocab axis L split across 8 cores (6250 cols
each), self_attn/value_w replicated, no communication. The embedding
shard is pre-transposed on host to [D=128, Lsh] so the contraction dim
sits on SBUF partitions.

Per-core Bass/Tile program:
  - DMA value_w as [T,1], self_attn as [T, N*D] (natural row layout).
  - 16 PE matmuls (lhsT=self_attn[n] [T,D], rhs=w [T,1]) accumulate
    ctxT columns -> PSUM [D, N] -> SBUF.
  - Main loop: lhsT=ctxT [D,16] stationary, rhs = emb chunks [D,<=512]
    streamed -> PSUM [16,<=512] -> DVE copy -> store DMA.
  - Loads go on the sync (SP) HWDGE ring, stores on the scalar (ACT)
    ring so stores don't queue behind remaining loads.
"""

import numpy as np

L = 50000
D = 128
T = 100
N = 16
NCORES = 8
LSH = L // NCORES          # 6250 columns per core

DMA_CHUNK = 1536           # emb load granularity (786 KB per DMA)
MM_CHUNK = 512             # fp32 matmul moving-operand / PSUM bank limit

_cache = {}


def _chunks(total, step):
    return [(c0, min(c0 + step, total)) for c0 in range(0, total, step)]


def _build():
    import concourse.bacc as bacc
    import concourse.mybir as mybir
    import concourse.tile as tile

    nc = bacc.Bacc(
        "TRN2",
        target_bir_lowering=False,
        debug=False,
        enable_asserts=True,
        num_devices=NCORES,
    )

    embT = nc.dram_tensor("embT", [D, LSH], mybir.dt.float32, kind="ExternalInput").ap()
    attn = nc.dram_tensor("attn", [N, T, D], mybir.dt.float32, kind="ExternalInput").ap()
    wv = nc.dram_tensor("wv", [1, T], mybir.dt.float32, kind="ExternalInput").ap()
    out = nc.dram_tensor("out", [N, LSH], mybir.dt.float32, kind="ExternalOutput").ap()

    dma_chunks = _chunks(LSH, DMA_CHUNK)

    with tile.TileContext(nc) as tc:
        with (
            tc.tile_pool(name="consts", bufs=1) as consts,
            tc.tile_pool(name="embp", bufs=len(dma_chunks)) as embp,
            tc.tile_pool(name="outp", bufs=3) as outp,
            tc.tile_pool(name="psc", bufs=1, space="PSUM") as psc,
            tc.tile_pool(name="pso", bufs=4, space="PSUM") as pso,
        ):
            w_tile = consts.tile([T, 1], mybir.dt.float32)
            nc.sync.dma_start(w_tile[:, :], wv.rearrange("o t -> t o"))

            attn_tile = consts.tile([T, N * D], mybir.dt.float32)
            nc.sync.dma_start(
                attn_tile.rearrange("t (n d) -> t n d", n=N),
                attn.rearrange("n t d -> t n d"),
            )

            emb_tiles = []
            for c0, c1 in dma_chunks:
                et = embp.tile(
                    [D, c1 - c0], mybir.dt.float32, tag="emb", name=f"emb_{c0}"
                )
                nc.sync.dma_start(et[:, :], embT[:, c0:c1])
                emb_tiles.append(et)

            # ctxT[d, n] = sum_t self_attn[n, t, d] * w[t]
            ps_ctx = psc.tile([D, N], mybir.dt.float32)
            for n in range(N):
                nc.tensor.matmul(
                    ps_ctx[:, n : n + 1],
                    lhsT=attn_tile[:, n * D : (n + 1) * D],
                    rhs=w_tile[:, :],
                    start=True,
                    stop=True,
                )
            ctxT = consts.tile([D, N], mybir.dt.float32)
            nc.vector.tensor_copy(ctxT[:, :], ps_ctx[:, :])

            # out[n, c0:c1] = ctxT.T @ embT[:, c0:c1]
            for ci, (c0, c1) in enumerate(dma_chunks):
                ot = outp.tile([N, c1 - c0], mybir.dt.float32, tag="out", name=f"out_{c0}")
                for s0, s1 in _chunks(c1 - c0, MM_CHUNK):
                    ps = pso.tile(
                        [N, s1 - s0], mybir.dt.float32, tag="pso", name=f"ps_{c0}_{s0}"
                    )
                    nc.tensor.matmul(
                        ps[:, :],
                        lhsT=ctxT[:, :],
                        rhs=emb_tiles[ci][:, s0:s1],
                        start=True,
                        stop=True,
                    )
                    nc.vector.tensor_copy(ot[:, s0:s1], ps[:, :])
                nc.scalar.dma_start(out[:, c0:c1], ot[:, :])

    nc.compile()
    return nc


def _get_nc():
    if "nc" not in _cache:
        _cache["nc"] = _build()
    return _cache["nc"]


def _make_in_maps(self_attn, emb_table, value_w):
    self_attn = np.ascontiguousarray(np.asarray(self_attn, dtype=np.float32))
    value_w = np.ascontiguousarray(np.asarray(value_w, dtype=np.float32))
    embT = np.asarray(emb_table, dtype=np.float32)[1 : L + 1].T  # [D, L]
    return [
        {
            "embT": np.ascontiguousarray(embT[:, k * LSH : (k + 1) * LSH]),
            "attn": self_attn,
            "wv": value_w,
        }
        for k in range(NCORES)
    ]


def run(self_attn, emb_table, value_w, trace=False):
    from concourse.bass_utils import run_bass_kernel_spmd

    nc = _get_nc()
    in_maps = _make_in_maps(self_attn, emb_table, value_w)
    res = run_bass_kernel_spmd(nc, in_maps, list(range(NCORES)), trace=trace)
    full = np.concatenate(
        [res.results[k]["out"] for k in range(NCORES)], axis=1
    ).astype(np.float32)
    return full, res


def kernel(self_attn, mat2, traj, emb_table, value_w):
    full, _ = run(self_attn, emb_table, value_w, trace=False)
    return full
